# revision 1
# baseline (speedup 1.0000x reference)
"""Trainium2 kernel for nn_ClusteringLayer (vq_codebook).

Problem: x (1, 131072, 256) f32, cluster_centers (1024, 256) f32.
For each cluster k: find argmin_n ||x[n] - c[k]||^2 and return that x row.
Output: (1, 1024, 256) f32.

Strategy (8 NeuronCores, x sharded along n, centers replicated):
  argmin_n d2[n,k] == argmax_n s[n,k],  s = 2*x.c - |x|^2  (c2[k] const per k)
  Host pre-sorts points by |x|^2, so |x|^2 is nearly constant inside each
  contiguous 2048-point group. The device then needs no x2 at all:
    psum[k_tile, grp] = bf16 matmul  xT_sorted (moving) x (2C)T (stationary)
    VectorE reduce_max over each group directly from PSUM -> bmax2dot f32.
  Host recovery per cluster:
    upper/lower bounds of the true group max of s from bmax2dot and the
    group's [x2min, x2max]; every group whose upper bound reaches the best
    lower bound - THETA is rescored exactly (fp32 gemm + fp64 refine,
    first-original-index tiebreak). Exactness relies only on bounds +
    THETA covering the bf16 matmul noise (~0.12 abs, validated).
"""

import os
import sys

for _p in ("/opt/trn_rl_repo",):
    if os.path.isdir(_p) and _p not in sys.path:
        sys.path.append(_p)

import numpy as np
import ml_dtypes

import concourse.bass as bass
import concourse.bacc as bacc
import concourse.mybir as mybir
import concourse.tile as tile

NCORES = 8
N = 131072
F = 256
K = 1024
SH = N // NCORES            # 16384 points per core
GRP = 2048                  # group size for the device-side max reduction
NG = SH // GRP              # 8 groups per core
NGRP = NCORES * NG          # 64 groups total
KT = K // 128               # 8 cluster tiles
NCH = F // 128              # 2 contraction chunks
THETA = 2.5                 # host rescue radius (covers bf16 score noise)
TOPM = 32                   # fp32->fp64 refine width per (cluster, group)

BF16 = ml_dtypes.bfloat16


def build_nc():
    """Build + compile the per-core Bass program (same program on all cores)."""
    nc = bacc.Bacc("TRN2", target_bir_lowering=False, debug=False,
                   num_devices=NCORES)

    xt = nc.dram_tensor("xt", [NCH, 128, SH], mybir.dt.bfloat16,
                        kind="ExternalInput")
    ct2 = nc.dram_tensor("ct2", [NCH, 128, K], mybir.dt.bfloat16,
                         kind="ExternalInput")
    bmax_d = nc.dram_tensor("bmax", [128, KT * NG], mybir.dt.float32,
                            kind="ExternalOutput")

    with tile.TileContext(nc) as tc:
        with (
            tc.tile_pool(name="consts", bufs=1) as cpool,
            tc.tile_pool(name="xtp", bufs=3) as xpool,
            tc.tile_pool(name="psum", bufs=2, space="PSUM") as ppool,
            tc.tile_pool(name="scrap", bufs=3) as spool,
        ):
            warm_w = cpool.tile([128, 128], mybir.dt.bfloat16, tag="warmw")
            warm_x = cpool.tile([128, 512], mybir.dt.bfloat16, tag="warmx")
            nc.gpsimd.memset(warm_w[:], 0.0)
            nc.gpsimd.memset(warm_x[:], 0.0)
            warm_ps = ppool.tile([128, 512], mybir.dt.float32, tag="ps",
                                 name="warmps")
            for _ in range(24):
                nc.tensor.matmul(warm_ps[:], lhsT=warm_w[:], rhs=warm_x[:],
                                 start=True, stop=True)

            ct2_t = []
            for ch in range(NCH):
                t = cpool.tile([128, K], mybir.dt.bfloat16, tag=f"ct{ch}")
                for h in range(2):
                    nc.sync.dma_start(t[:, h * K // 2:(h + 1) * K // 2],
                                      ct2[ch, :, h * K // 2:(h + 1) * K // 2])
                ct2_t.append(t)
            bmax_t = cpool.tile([128, KT * NG], mybir.dt.float32, tag="bmax")

            for g in range(NG):
                # per-512-block x tiles: finer DMA granularity lets the first
                # matmuls start as soon as one 128KB slice lands
                xg = []
                for ch in range(NCH):
                    blks = []
                    for blk in range(GRP // 512):
                        t = xpool.tile([128, 512], mybir.dt.bfloat16,
                                       tag=f"xt{ch}b{blk}")
                        base = g * GRP + blk * 512
                        nc.sync.dma_start(t[:], xt[ch, :, base:base + 512])
                        blks.append(t)
                    xg.append(blks)

                for kt in range(KT):
                    ps = ppool.tile([128, GRP], mybir.dt.float32, tag="ps")
                    for ch in range(NCH):
                        for blk in range(GRP // 512):
                            nc.tensor.matmul(
                                ps[:, blk * 512:(blk + 1) * 512],
                                lhsT=ct2_t[ch][:, kt * 128:(kt + 1) * 128],
                                rhs=xg[ch][blk][:],
                                start=(ch == 0),
                                stop=(ch == NCH - 1),
                            )
                    if True:
                        col = kt * NG + g
                        # ~1 direct-psum reduce per group, rest evac+fold
                        if kt == (g % KT):
                            nc.vector.tensor_reduce(
                                out=bmax_t[:, col:col + 1],
                                in_=ps[:],
                                axis=mybir.AxisListType.X,
                                op=mybir.AluOpType.max,
                            )
                        else:
                            ev = spool.tile([128, GRP], mybir.dt.float16,
                                            tag="ev")
                            nc.scalar.copy(ev[:], ps[:])
                            f1 = spool.tile([128, GRP // 2], mybir.dt.float16,
                                            tag="f1")
                            nc.vector.tensor_tensor(
                                out=f1[:], in0=ev[:, 0:GRP // 2],
                                in1=ev[:, GRP // 2:GRP],
                                op=mybir.AluOpType.max)
                            f2 = spool.tile([128, GRP // 4], mybir.dt.float16,
                                            tag="f2")
                            nc.vector.tensor_tensor(
                                out=f2[:], in0=f1[:, 0:GRP // 4],
                                in1=f1[:, GRP // 4:GRP // 2],
                                op=mybir.AluOpType.max)
                            f3 = spool.tile([128, GRP // 8], mybir.dt.float16,
                                            tag="f3")
                            nc.vector.tensor_tensor(
                                out=f3[:], in0=f2[:, 0:GRP // 8],
                                in1=f2[:, GRP // 8:GRP // 4],
                                op=mybir.AluOpType.max)
                            nc.vector.tensor_reduce(
                                out=bmax_t[:, col:col + 1],
                                in_=f3[:],
                                axis=mybir.AxisListType.X,
                                op=mybir.AluOpType.max,
                            )

            nc.sync.dma_start(bmax_d[:, :], bmax_t[:])

    nc.compile()
    return nc


def host_prep(x, cluster_centers):
    """Sort points by |x|^2; build per-core device inputs."""
    x0 = np.ascontiguousarray(x[0], dtype=np.float32)        # (N, F)
    C = np.ascontiguousarray(cluster_centers, dtype=np.float32)
    x2 = np.einsum('nf,nf->n', x0.astype(np.float64),
                   x0.astype(np.float64))
    order = np.argsort(x2, kind="stable").astype(np.int64)
    xs_all = x0[order]                                        # sorted points
    x2s = x2[order]
    ct2_np = np.ascontiguousarray(
        (2.0 * C).T.astype(BF16)).reshape(NCH, 128, K)
    in_maps = []
    for c in range(NCORES):
        xs = xs_all[c * SH:(c + 1) * SH]
        xt_np = np.ascontiguousarray(xs.T.astype(BF16)).reshape(NCH, 128, SH)
        in_maps.append({"xt": xt_np, "ct2": ct2_np})
    return in_maps, x0, C, order, xs_all, x2s


def host_combine(bmax_cores, x0, C, order, xs_all, x2s):
    """Exact argmin recovery from per-group maxima of 2*dot (sorted points)."""
    x64s = xs_all.astype(np.float64)
    C64 = C.astype(np.float64)
    x2s_32 = x2s.astype(np.float32)

    # bmax_cores[c]: [128, KT*NG] -> cluster k = kt*128 + p, col = kt*NG + g
    bm = np.empty((K, NGRP), dtype=np.float32)
    for c in range(NCORES):
        a = np.asarray(bmax_cores[c]).reshape(128, KT, NG)
        bm[:, c * NG:(c + 1) * NG] = a.transpose(1, 0, 2).reshape(K, NG)

    gb = np.arange(NGRP) * GRP
    x2min = x2s[gb].astype(np.float32)            # sorted -> min is first
    x2max = x2s[gb + GRP - 1].astype(np.float32)

    ub = bm - x2min[None, :]                      # >= true group smax
    lb = bm - x2max[None, :]                      # <= true group smax
    win_lb = lb.max(axis=1)
    flags = ub >= (win_lb[:, None] - THETA)       # (K, NGRP)

    pair_clusters = [[] for _ in range(NGRP)]
    ks_idx, ps_idx = np.nonzero(flags)
    for kk, p in zip(ks_idx, ps_idx):
        pair_clusters[p].append(kk)

    best_val = np.full(K, np.inf)
    best_idx = np.zeros(K, dtype=np.int64)        # original indices
    for p, ks in enumerate(pair_clusters):
        if not ks:
            continue
        base = p * GRP
        pts = xs_all[base:base + GRP]
        d32 = x2s_32[base:base + GRP, None] - 2.0 * (pts @ C[ks].T)
        m = min(TOPM, GRP - 1)
        part = np.argpartition(d32, m, axis=0)[:m]
        for j, kk in enumerate(ks):
            srt = base + part[:, j]
            dv = x2s[srt] - 2.0 * (x64s[srt] @ C64[kk])
            ids = order[srt]                      # original indices
            o = np.lexsort((ids, dv))[0]
            if (dv[o] < best_val[kk]) or (dv[o] == best_val[kk]
                                          and ids[o] < best_idx[kk]):
                best_val[kk] = dv[o]
                best_idx[kk] = ids[o]

    return x0[best_idx][None].astype(np.float32)


_NC_CACHE = {}


def kernel(x, cluster_centers):
    from concourse.bass_utils import run_bass_kernel_spmd

    if "nc" not in _NC_CACHE:
        _NC_CACHE["nc"] = build_nc()
    nc = _NC_CACHE["nc"]

    in_maps, x0, C, order, xs_all, x2s = host_prep(x, cluster_centers)
    res = run_bass_kernel_spmd(nc, in_maps, list(range(NCORES)))
    bmax_cores = [res.results[c]["bmax"] for c in range(NCORES)]
    return host_combine(bmax_cores, x0, C, order, xs_all, x2s)



# revision 5
# speedup vs baseline: 1.4551x; 1.4551x over previous
"""Trainium2 kernel for nn_ClusteringLayer (vq_codebook).

Problem: x (1, 131072, 256) f32, cluster_centers (1024, 256) f32.
For each cluster k: find argmin_n ||x[n] - c[k]||^2 and return that x row.
Output: (1, 1024, 256) f32.

Strategy (8 NeuronCores, x sharded along n, centers replicated):
  argmin_n d2[n,k] == argmax_n s[n,k],  s = 2*x.c - |x|^2  (c2[k] const per k)
  Host pre-sorts points by |x|^2 so |x|^2 is nearly constant inside each
  contiguous 2048-point group; the device screens on dot = 2*x.c only.

  v2 (fp8): the N x K dot screen runs as fp8e4 DoubleRow matmuls
  (256-deep contraction in one pass, ~1.5x bf16 throughput).  The PSUM
  group reduction is split across BOTH free engines as single-pass
  reducers, no fold traffic:
    - VectorE:  tensor_reduce(max) straight from PSUM -> bmax2dot
    - ScalarE:  activation(Exp, accum_out) straight from PSUM -> sum of
      exp(T*(s - M_k)), a log-sum-exp sketch of the same group max:
        M_k + ln(acc)/T  >= group max >= M_k + ln(acc)/T - ln(2048)/T
  Host recovery per cluster: upper/lower bounds of the true group max of
  s from either sketch plus the group's [x2min, x2max]; every group whose
  upper bound reaches the best lower bound is rescored exactly (fp32 gemm
  + fp64 refine, first-original-index tiebreak).  THETA covers the fp8
  matmul noise (measured max ~5.9 on 400k samples, ~7.4 predicted tail
  over all 134M scores; THETA=11).
"""

import os
import sys

for _p in ("/opt/trn_rl_repo",):
    if os.path.isdir(_p) and _p not in sys.path:
        sys.path.append(_p)

import numpy as np
import ml_dtypes

import concourse.bass as bass
import concourse.bacc as bacc
import concourse.mybir as mybir
import concourse.tile as tile

NCORES = 8
N = 131072
F = 256
K = 1024
SH = N // NCORES            # 16384 points per core
GRP = 1024                  # group size for the device-side reduction
NG = SH // GRP              # 8 groups per core
NGRP = NCORES * NG          # 64 groups total
KT = K // 128               # 8 cluster tiles
NUNIT = NG * KT             # 64 (group, ktile) units per core
THETA = 11.0                # host rescue radius (covers fp8 score noise)
T_LSE = 1.0                 # LSE sharpness; lb gap = ln(1024)/T ~ 6.9
LOG_DENORM = 103.28         # -ln(smallest f32 denormal): acc==0 max-bound
TOPM = 32                   # fp32->fp64 refine width per (cluster, group)
NV = 69                     # units on VectorE (rest on ScalarE LSE)

E4 = ml_dtypes.float8_e4m3fn

# unit u (execution order: u = g*KT + kt) -> True if VectorE handles it
UNIT_IS_V = [(u * NV) // NUNIT != ((u + 1) * NV) // NUNIT for u in range(NUNIT)]


def build_nc():
    """Build + compile the per-core Bass program (same program on all cores)."""
    nc = bacc.Bacc("TRN2", target_bir_lowering=False, debug=False,
                   num_devices=NCORES)

    xt = nc.dram_tensor("xt", [128, 2, SH], mybir.dt.float8e4,
                        kind="ExternalInput")
    ct = nc.dram_tensor("ct", [128, 2 * KT, 128], mybir.dt.float8e4,
                        kind="ExternalInput")
    bias = nc.dram_tensor("bias", [128, KT], mybir.dt.float32,
                          kind="ExternalInput")
    outv_d = nc.dram_tensor("outv", [128, NUNIT], mybir.dt.float32,
                            kind="ExternalOutput")
    outs_d = nc.dram_tensor("outs", [128, NUNIT], mybir.dt.float32,
                            kind="ExternalOutput")

    DR = mybir.MatmulPerfMode.DoubleRow

    with tile.TileContext(nc) as tc:
        with (
            tc.tile_pool(name="consts", bufs=1) as cpool,
            tc.tile_pool(name="xtp", bufs=1) as xpool,
            tc.tile_pool(name="psum", bufs=4, space="PSUM") as ppool,
            tc.tile_pool(name="scrap", bufs=2) as spool,
        ):
            ct_t = cpool.tile([128, 2 * KT, 128], mybir.dt.float8e4, tag="ct")
            nc.sync.dma_start(ct_t[:], ct[:, :, :])
            b_t = cpool.tile([128, KT], mybir.dt.float32, tag="b")
            nc.sync.dma_start(b_t[:], bias[:, :])

            # prefetch the whole shard (fp8: 4KB/partition per group chunk)
            xg = []
            for g in range(NG):
                t = xpool.tile([128, 2, GRP], mybir.dt.float8e4, tag=f"x{g}")
                nc.sync.dma_start(t[:], xt[:, :, g * GRP:(g + 1) * GRP])
                xg.append(t)

            # HAM warmup while DMAs land (aliases the main psum tag)
            warm_w = cpool.tile([128, 128], mybir.dt.bfloat16, tag="warmw")
            warm_x = cpool.tile([128, 512], mybir.dt.bfloat16, tag="warmx")
            nc.gpsimd.memset(warm_w[:], 0.0)
            nc.gpsimd.memset(warm_x[:], 0.0)
            warm_ps = ppool.tile([128, 512], mybir.dt.float32, tag="ps",
                                 name="warmps")
            for _ in range(16):
                nc.tensor.matmul(warm_ps[:], lhsT=warm_w[:], rhs=warm_x[:],
                                 start=True, stop=True)

            outv_t = cpool.tile([128, NUNIT], mybir.dt.float32, tag="ov")
            outs_t = cpool.tile([128, NUNIT], mybir.dt.float32, tag="os")

            for g in range(NG):
                for kt in range(KT):
                    u = g * KT + kt
                    ps = ppool.tile([128, GRP], mybir.dt.float32, tag="ps",
                                    name=f"ps{u}")
                    for b in range(GRP // 512):
                        nc.tensor.matmul(
                            ps[:, b * 512:(b + 1) * 512],
                            lhsT=ct_t[:, 2 * kt:2 * kt + 2, :],
                            rhs=xg[g][:, :, b * 512:(b + 1) * 512],
                            start=True, stop=True, perf_mode=DR,
                        )
                    if UNIT_IS_V[u]:
                        nc.vector.tensor_reduce(
                            out=outv_t[:, u:u + 1], in_=ps[:],
                            axis=mybir.AxisListType.X,
                            op=mybir.AluOpType.max,
                        )
                    else:
                        scr = spool.tile([128, GRP], mybir.dt.bfloat16,
                                         tag="scr")
                        nc.scalar.activation(
                            scr[:], ps[:], mybir.ActivationFunctionType.Exp,
                            bias=b_t[:, kt:kt + 1], scale=T_LSE,
                            accum_out=outs_t[:, u:u + 1],
                        )

            nc.sync.dma_start(outv_d[:, :], outv_t[:])
            nc.sync.dma_start(outs_d[:, :], outs_t[:])

    nc.compile()
    return nc


def host_prep(x, cluster_centers):
    """Sort points by |x|^2; build per-core fp8 device inputs."""
    x0 = np.ascontiguousarray(x[0], dtype=np.float32)        # (N, F)
    C = np.ascontiguousarray(cluster_centers, dtype=np.float32)
    x2 = np.einsum('nf,nf->n', x0.astype(np.float64),
                   x0.astype(np.float64))
    order = np.argsort(x2, kind="stable").astype(np.int64)
    xs_all = x0[order]                                        # sorted points
    x2s = x2[order]

    # ct[p, 2*kt+i, m] = 2*C[kt*128+m, i*128+p]
    T = (2.0 * C).reshape(KT, 128, 2, 128)                    # [kt, m, i, p]
    ct_np = np.ascontiguousarray(
        T.transpose(3, 0, 2, 1).reshape(128, 2 * KT, 128).astype(E4))

    # bias[p, kt] = -T * M_k, M_k a safe upper bound of max_n 2x.c_k
    cn = np.linalg.norm(C, axis=1)                            # (K,)
    M = 2.0 * cn * 4.9 + 10.0
    bias_np = np.ascontiguousarray(
        (-T_LSE * M).reshape(KT, 128).T.astype(np.float32))   # [p, kt]

    in_maps = []
    for c in range(NCORES):
        xs = xs_all[c * SH:(c + 1) * SH]                      # (SH, F)
        xt_np = np.ascontiguousarray(
            xs.T.reshape(2, 128, SH).transpose(1, 0, 2).astype(E4))
        in_maps.append({"xt": xt_np, "ct": ct_np, "bias": bias_np})
    return in_maps, x0, C, order, xs_all, x2s, M


def host_combine(res_cores, x0, C, order, xs_all, x2s, M):
    """Exact argmin recovery from per-(group,ktile) max sketches."""
    x64s = xs_all.astype(np.float64)
    C64 = C.astype(np.float64)
    x2s_32 = x2s.astype(np.float32)

    # build ub/lb on max_n 2x.c per (cluster k, global group p = c*NG+g)
    smax_ub = np.empty((K, NGRP), dtype=np.float64)
    smax_lb = np.empty((K, NGRP), dtype=np.float64)
    lgap = np.log(float(GRP)) / T_LSE
    for c in range(NCORES):
        outv, outs = res_cores[c]                             # [128, NUNIT]
        for g in range(NG):
            p = c * NG + g
            for kt in range(KT):
                u = g * KT + kt
                ks = slice(kt * 128, (kt + 1) * 128)
                if UNIT_IS_V[u]:
                    bm = outv[:, u].astype(np.float64)
                    smax_ub[ks, p] = bm + THETA
                    smax_lb[ks, p] = bm - THETA
                else:
                    acc = outs[:, u].astype(np.float64)
                    Mk = M[ks].astype(np.float64)
                    with np.errstate(divide='ignore'):
                        l = np.log(np.maximum(acc, 0.0))
                    bad = ~np.isfinite(acc) | (acc < 0)
                    lse = Mk + l / T_LSE
                    lse[bad] = np.inf
                    ub = lse + THETA
                    # acc underflowed to 0: true group max still can be as
                    # high as M - ln(1/denorm_min)/T -- not -inf
                    zero = ~bad & (acc <= 0)
                    ub[zero] = Mk[zero] - LOG_DENORM / T_LSE + THETA
                    smax_ub[ks, p] = ub
                    lb = lse - lgap - THETA
                    lb[bad | (acc <= 0)] = -np.inf
                    smax_lb[ks, p] = lb

    gb = np.arange(NGRP) * GRP
    x2min = x2s[gb]
    x2max = x2s[gb + GRP - 1]

    ub = smax_ub - x2min[None, :]                 # >= true group max of -d2
    lb = smax_lb - x2max[None, :]                 # <= true group max of -d2
    win_lb = lb.max(axis=1)
    flags = ub >= win_lb[:, None]                 # (K, NGRP)

    pair_clusters = [[] for _ in range(NGRP)]
    ks_idx, ps_idx = np.nonzero(flags)
    for kk, p in zip(ks_idx, ps_idx):
        pair_clusters[p].append(kk)

    best_val = np.full(K, np.inf)
    best_idx = np.zeros(K, dtype=np.int64)        # original indices
    for p, ks in enumerate(pair_clusters):
        if not ks:
            continue
        base = p * GRP
        pts = xs_all[base:base + GRP]
        d32 = x2s_32[base:base + GRP, None] - 2.0 * (pts @ C[ks].T)
        m = min(TOPM, GRP - 1)
        part = np.argpartition(d32, m, axis=0)[:m]
        for j, kk in enumerate(ks):
            srt = base + part[:, j]
            dv = x2s[srt] - 2.0 * (x64s[srt] @ C64[kk])
            ids = order[srt]                      # original indices
            o = np.lexsort((ids, dv))[0]
            if (dv[o] < best_val[kk]) or (dv[o] == best_val[kk]
                                          and ids[o] < best_idx[kk]):
                best_val[kk] = dv[o]
                best_idx[kk] = ids[o]

    return x0[best_idx][None].astype(np.float32)


_NC_CACHE = {}


def kernel(x, cluster_centers):
    from concourse.bass_utils import run_bass_kernel_spmd

    if "nc" not in _NC_CACHE:
        _NC_CACHE["nc"] = build_nc()
    nc = _NC_CACHE["nc"]

    in_maps, x0, C, order, xs_all, x2s, M = host_prep(x, cluster_centers)
    res = run_bass_kernel_spmd(nc, in_maps, list(range(NCORES)))
    res_cores = [(res.results[c]["outv"], res.results[c]["outs"])
                 for c in range(NCORES)]
    return host_combine(res_cores, x0, C, order, xs_all, x2s, M)
